# revision 27
# baseline (speedup 1.0000x reference)
"""Trainium2 Bass kernel for a GPT-style transformer block (B=2, T=2048, C=768,
NH=12, HD=64, DFF=3072), distributed over 8 NeuronCores.

Sharding: token-data-parallel with zigzag strip assignment, zero collectives.
  - cores 0-3 process batch 0, cores 4-7 batch 1.
  - within a batch, rank r owns token strips r and 7-r (strips of 256 tokens).
  - each core redundantly computes K/V for tokens [0, 256*(8-r)) (its causal
    prefix), so no cross-core communication is needed at all.

All GEMM operands are bf16 (cast on host; fp32 PSUM accumulation), which
halves HBM traffic and SBUF footprint and removes all on-device weight casts.
One pass over all 12 heads; LN1+transpose is software-pipelined with the K/V
GEMMs per 512-token super-block. Attention softmax uses exp-without-max in
large batched ACT instructions with the normalization folded into the PSUM
eviction (ones-column trick).
"""

import sys
import types
import functools

sys.path.insert(0, "/opt/trn_rl_repo")

# ---- antenv.axon_hooks shim (missing module in this image) -----------------
if "antenv.axon_hooks" not in sys.modules:
    _hooks = types.ModuleType("antenv.axon_hooks")
    _hooks._hook = None
    _hooks.set_axon_ntff_profile_hook = lambda h: setattr(_hooks, "_hook", h)
    _hooks.get_axon_ntff_profile_hook = lambda: _hooks._hook
    sys.modules["antenv.axon_hooks"] = _hooks
    try:
        import antenv

        antenv.axon_hooks = _hooks
    except ImportError:
        pass

import numpy as np
import jax

import concourse.bass as bass
import concourse.mybir as mybir
import concourse.tile as tile
from concourse import bacc
from concourse.bass2jax import (
    _bass_exec_p,
    install_neuronx_cc_hook,
    partition_id_tensor,
)
from concourse.masks import make_identity

B, T, C = 2, 2048, 768
NH, HD, DFF = 12, 64, 64 * 48  # DFF = 3072
F32 = mybir.dt.float32
BF16 = mybir.dt.bfloat16
EPS = 1e-5


# ---------------------------------------------------------------------------
# Per-rank program builder
# ---------------------------------------------------------------------------
def build_rank_program(r: int, with_bias: bool):
    """Program for rank r (strips r and 7-r of one batch element)."""
    sA, sB = r, 7 - r
    NTK = 2 * (8 - r)          # kt tiles of 128 in the causal prefix
    T_kv = NTK * 128

    nc = bacc.Bacc("TRN2", target_bir_lowering=False, debug=False, num_devices=1)

    xb_in = nc.declare_dram_parameter("xb", [T_kv, C], BF16, isOutput=False)
    xo_in = nc.declare_dram_parameter("xo", [512, C], F32, isOutput=False)
    wq_in = nc.declare_dram_parameter("wq", [C, C], BF16, isOutput=False)
    wk_in = nc.declare_dram_parameter("wk", [C, C], BF16, isOutput=False)
    wv_in = nc.declare_dram_parameter("wv", [C, C], BF16, isOutput=False)
    wcp_in = nc.declare_dram_parameter("wcp", [C, C], BF16, isOutput=False)
    wfc_in = nc.declare_dram_parameter("wfc", [C, DFF], BF16, isOutput=False)
    wpj_in = nc.declare_dram_parameter("wpj", [DFF, C], BF16, isOutput=False)
    bias_ins = {}
    if with_bias:
        for nm, sz in (("bq", C), ("bk", C), ("bv", C), ("bcp", C),
                       ("bfc", DFF), ("bpj", C)):
            bias_ins[nm] = nc.declare_dram_parameter(nm, [sz], F32, isOutput=False)
    out_dram = nc.declare_dram_parameter("out", [512, C], F32, isOutput=True)

    with tile.TileContext(nc) as tc:
        _build_body(nc, tc, r, sA, sB, NTK, T_kv,
                    xb_in, xo_in, wq_in, wk_in, wv_in, wcp_in, wfc_in, wpj_in,
                    bias_ins, out_dram)
    nc.compile()
    return nc


def _build_body(nc, tc, r, sA, sB, NTK, T_kv,
                xb_in, xo_in, wq_in, wk_in, wv_in, wcp_in, wfc_in, wpj_in,
                bias_ins, out_dram):
    from contextlib import ExitStack

    WB = bool(bias_ins)
    n_sh = 2 * (sA + 1)            # kt chunks attended by both strips
    n_all = 2 * (sB + 1)           # kt chunks attended by strip B ( == NTK )
    assert n_all == NTK

    def strided2(base_ap, tbA, tbB, w):
        """Columns [tbA:tbA+w] and [tbB:tbB+w] of a [128, T] AP as [2, w]."""
        stride = base_ap.ap[-1][0]
        return bass.AP(
            tensor=base_ap.tensor,
            offset=base_ap[:, tbA:tbA + 1].offset,
            ap=[list(p) for p in base_ap.ap[:1]]
            + [[stride * (tbB - tbA), 2], [stride, w]],
        )

    with ExitStack() as ctx:
        # ------- constants -------
        const = ctx.enter_context(tc.tile_pool(name="const", bufs=1))
        id_f = const.tile([128, 128], F32)
        make_identity(nc, id_f[:])
        id_b = const.tile([128, 128], BF16)
        nc.vector.tensor_copy(id_b[:], id_f[:])
        eps_t = const.tile([128, 1], F32)
        nc.vector.memset(eps_t[:], EPS)
        # causal masks for the two in-strip kt chunk offsets: [128, 2, 256]
        mask_f = const.tile([128, 2, 256], F32)
        nc.vector.memset(mask_f[:], 1.0)
        for off in range(2):
            nc.gpsimd.affine_select(
                out=mask_f[:, off, :],
                in_=mask_f[:, off, :],
                compare_op=mybir.AluOpType.is_ge,
                fill=0.0,
                base=-128 * off,
                pattern=[[1, 256]],
                channel_multiplier=-1,
            )
        mask_t = const.tile([128, 2, 256], BF16)
        nc.vector.tensor_copy(mask_t[:], mask_f[:])

        if WB:
            bq_sb = const.tile([128, 6], F32)
            bk_sb = const.tile([128, 6], F32)
            for src, dst in ((bias_ins["bq"], bq_sb), (bias_ins["bk"], bk_sb)):
                nc.sync.dma_start(out=dst[:], in_=src[:].rearrange("(j p) -> p j", p=128))
            bfc_sb = const.tile([128, 24], F32)
            nc.sync.dma_start(out=bfc_sb[:], in_=bias_ins["bfc"][:].rearrange("(f p) -> p f", p=128))
            brow_f = const.tile([1, 3, C], F32)
            nc.sync.dma_start(out=brow_f[:, 0, :], in_=bias_ins["bv"][:][None, :])
            nc.sync.dma_start(out=brow_f[:, 1, :], in_=bias_ins["bcp"][:][None, :])
            nc.sync.dma_start(out=brow_f[:, 2, :], in_=bias_ins["bpj"][:][None, :])
            bias_bc = const.tile([128, 3, C], F32)
            nc.gpsimd.partition_broadcast(bias_bc[:], brow_f[:])
            bv_bc = bias_bc[:, 0, :]
            bcp_bc = bias_bc[:, 1, :]
            bpj_bc = bias_bc[:, 2, :]

        # ------- activation tensors spanning attention + MLP -------
        acts = ctx.enter_context(tc.tile_pool(name="acts", bufs=1))
        kT_sb = acts.tile([128, 6, T_kv], BF16)        # K^T, 12 heads
        v_sb = acts.tile([128, NTK, 12, 65], BF16)     # V natural + ones col
        qT_sb = acts.tile([128, 6, 512], BF16)         # Q^T for own strips
        yT_sb = acts.tile([128, 6, 512], BF16)         # attention out (normed)
        x1_sb = acts.tile([128, 4, C], F32)            # attn residual output
        h2T_sb = acts.tile([128, 6, 512], BF16)        # ln2 transposed
        gT_sb = acts.tile([128, 24, 512], BF16)        # gelu(fc) transposed
        wcp_sb = acts.tile([128, 6, C], BF16)          # c_proj weights

        nc.vector.memset(v_sb[:, :, :, 64:65], 1.0)

        # =========== stage 1+2: LN1 + transpose + K/V/Q GEMMs ==============
        # wv/hT (and the pv PSUM pool) outlive s12: V GEMMs for kt tiles >= 4
        # are deferred into attention pass 1 to fill its ACT-bound PE gaps.
        w12b = ExitStack()
        wvp = w12b.enter_context(tc.tile_pool(name="wvp", bufs=1))
        wv_sb = wvp.tile([128, 6, C], BF16)
        hT_sb = wvp.tile([128, 6, T_kv], BF16)
        pv_ps = w12b.enter_context(tc.tile_pool(name="pv_ps", bufs=1, space="PSUM"))

        s12 = ExitStack()
        w12 = s12.enter_context(tc.tile_pool(name="w12", bufs=1))
        wk_sb = w12.tile([128, 6, C], BF16)
        wq_sb = w12.tile([128, 6, C], BF16)
        # x tiles for the first super-block are needed immediately; weight
        # DMAs go behind them in the queue.
        xpre_pool = s12.enter_context(tc.tile_pool(name="xpre", bufs=1))
        xpre = xpre_pool.tile([128, 4, C], BF16)
        for tt in range(min(4, T_kv // 128)):
            nc.sync.dma_start(out=xpre[:, tt, :],
                              in_=xb_in[tt * 128:(tt + 1) * 128, :])
        for src, dst in ((wk_in, wk_sb), (wv_in, wv_sb), (wq_in, wq_sb),
                         (wcp_in, wcp_sb)):
            nc.sync.dma_start(out=dst[:], in_=src[:].rearrange("(c k) n -> k c n", k=128))

        ln_pool = s12.enter_context(tc.tile_pool(name="ln", bufs=3))
        gemm_ps = s12.enter_context(tc.tile_pool(name="gemm_ps", bufs=2, space="PSUM"))
        tp_ps = s12.enter_context(tc.tile_pool(name="tp_ps", bufs=2, space="PSUM"))

        # super-blocks of up to 512 tokens
        sblocks = []
        t0 = 0
        while t0 < T_kv:
            w = min(512, T_kv - t0)
            sblocks.append((t0, w))
            t0 += w

        def v_gemm_ti(ti):
            for half in range(2):
                h0 = half * 6
                pv = pv_ps.tile([128, 384], F32, tag=f"pv{half}")
                for c in range(6):
                    nc.tensor.matmul(
                        pv[:], hT_sb[:, c, ti * 128:(ti + 1) * 128],
                        wv_sb[:, c, half * 384:(half + 1) * 384],
                        start=(c == 0), stop=(c == 5),
                    )
                vdst = v_sb[:, ti, h0:h0 + 6, 0:64]
                if WB:
                    nc.vector.tensor_tensor(
                        out=vdst,
                        in0=pv[:].rearrange("p (h d) -> p h d", d=64),
                        in1=bv_bc[:, h0 * 64:(h0 + 6) * 64].rearrange(
                            "p (h d) -> p h d", d=64),
                        op=mybir.AluOpType.add,
                    )
                else:
                    nc.vector.tensor_copy(
                        vdst, pv[:].rearrange("p (h d) -> p h d", d=64))

        for (tb, bw) in sblocks:
            ntile = bw // 128
            for tt in range(ntile):
                ti = tb // 128 + tt
                if ti < 4:
                    x_t = xpre[:, ti, :]
                else:
                    x_t = ln_pool.tile([128, C], BF16, tag="x")
                    nc.sync.dma_start(
                        out=x_t[:],
                        in_=xb_in[ti * 128:(ti + 1) * 128, :])
                xg = x_t.rearrange("p (g d) -> p g d", g=3)
                stats = ln_pool.tile([128, 3, 6], F32, tag="st")
                for g in range(3):
                    nc.vector.bn_stats(out=stats[:, g, :], in_=xg[:, g, :])
                mv = ln_pool.tile([128, 2], F32, tag="mv")
                nc.vector.bn_aggr(out=mv[:], in_=stats[:])
                rstd = ln_pool.tile([128, 1], F32, tag="rstd")
                nc.scalar.activation(
                    out=rstd[:], in_=mv[:, 1:2],
                    func=mybir.ActivationFunctionType.Sqrt,
                    bias=eps_t[:], scale=1.0,
                )
                nc.vector.reciprocal(out=rstd[:], in_=rstd[:])
                h_t = ln_pool.tile([128, C], BF16, tag="h")
                nc.vector.tensor_scalar(
                    out=h_t[:], in0=x_t[:],
                    scalar1=mv[:, 0:1], scalar2=rstd[:],
                    op0=mybir.AluOpType.subtract, op1=mybir.AluOpType.mult,
                )
                pt = tp_ps.tile([128, 6, 128], BF16, tag="tp")
                for c in range(6):
                    nc.tensor.transpose(pt[:, c, :], h_t[:, c * 128:(c + 1) * 128], id_b[:])
                # evict all 6 transposed chunks in one ACT copy (DVE is busy)
                nc.scalar.copy(
                    hT_sb[:, :, ti * 128:(ti + 1) * 128], pt[:],
                )
            # K GEMM for this super-block: kT[:, jj, tb:tb+bw]
            for jj in range(6):
                pk = gemm_ps.tile([128, 512], F32, tag="pk")
                for c in range(6):
                    nc.tensor.matmul(
                        pk[:, 0:bw], wk_sb[:, c, jj * 128:(jj + 1) * 128],
                        hT_sb[:, c, tb:tb + bw],
                        start=(c == 0), stop=(c == 5),
                    )
                if WB:
                    nc.vector.tensor_scalar(
                        out=kT_sb[:, jj, tb:tb + bw], in0=pk[:, 0:bw],
                        scalar1=bk_sb[:, jj:jj + 1], scalar2=None,
                        op0=mybir.AluOpType.add,
                    )
                else:
                    nc.vector.tensor_copy(kT_sb[:, jj, tb:tb + bw], pk[:, 0:bw])
            # V GEMM (natural layout) — only the first 4 kt tiles here; the
            # rest is issued inside attention pass 1 to fill PE gaps.
            for tt in range(ntile):
                ti = tb // 128 + tt
                if ti < 4:
                    v_gemm_ti(ti)

        # Q GEMM for own strips (both strips in one N=512 matmul)
        tbA, tbB = sA * 256, sB * 256
        for jj in range(6):
            pq = gemm_ps.tile([128, 512], F32, tag="pk")
            for c in range(6):
                rhs = strided2(hT_sb[:, c, :], tbA, tbB, 256)
                nc.tensor.matmul(
                    pq[:], wq_sb[:, c, jj * 128:(jj + 1) * 128], rhs,
                    start=(c == 0), stop=(c == 5),
                )
            if WB:
                nc.vector.tensor_scalar(
                    out=qT_sb[:, jj, :], in0=pq[:],
                    scalar1=bq_sb[:, jj:jj + 1], scalar2=None,
                    op0=mybir.AluOpType.add,
                )
            else:
                nc.vector.tensor_copy(qT_sb[:, jj, :], pq[:])

        s12.close()  # free wk/wq/ln SBUF (wv/hT/pv stay for deferred V)

        # ======================= stage 3: attention ========================
        # Two k-range passes with partial-softmax accumulation: pass 1 covers
        # kt chunks [0, 4) (whose K/V are ready first); the V GEMMs for kt
        # tiles >= 4 are interleaved between pass-1 heads so they fill the
        # ACT-bound PE gaps.  Pass 2 covers chunks [4, n_all) and merges with
        # the pass-1 partial sums held in an SBUF accumulator.
        C1 = 4
        p2sh = n_sh > C1            # pass 2 contains shared (full-width) chunks
        s3 = ExitStack()
        att_pool = s3.enter_context(tc.tile_pool(name="att", bufs=4))
        nrm_pool = s3.enter_context(tc.tile_pool(name="nrm", bufs=2))
        acc_pool = s3.enter_context(tc.tile_pool(name="acc", bufs=1))
        yt_acc = acc_pool.tile([65, 12, 512], F32)
        pa_ps = s3.enter_context(tc.tile_pool(name="pa_ps", bufs=2, space="PSUM"))
        yt_ps = s3.enter_context(tc.tile_pool(name="yt_ps", bufs=2, space="PSUM"))

        PIPE_AV = 5
        pending = []   # (kc, at_slice, qs, ww, yt, h, start, stop, action)

        def evict_pass1(yt, h):
            nc.vector.tensor_copy(yt_acc[:, h, :], yt[0:65, :])

        def finalize_head(yt, h):
            j, po = h // 2, 64 * (h % 2)
            acc = yt_acc[:, h, :]
            sume = nrm_pool.tile([1, 512], F32, tag="sume")
            num = nrm_pool.tile([64, 512], F32, tag="num")
            if p2sh:
                nc.vector.tensor_add(sume[:], yt[64:65, :], acc[64:65, :])
                nc.vector.tensor_add(num[:], yt[0:64, :], acc[0:64, :])
            else:
                nc.vector.tensor_copy(sume[:, 0:256], acc[64:65, 0:256])
                nc.vector.tensor_add(
                    sume[:, 256:512], yt[64:65, 256:512], acc[64:65, 256:512])
                nc.vector.tensor_copy(num[:, 0:256], acc[0:64, 0:256])
                nc.vector.tensor_add(
                    num[:, 256:512], yt[0:64, 256:512], acc[0:64, 256:512])
            bcast = nrm_pool.tile([64, 512], F32, tag="bcast")
            nc.gpsimd.partition_broadcast(bcast[:], sume[:])
            nc.vector.reciprocal_approx_fast(out=bcast[:], in_=bcast[:])
            nc.vector.tensor_mul(
                yT_sb[po:po + 64, j, :], num[:], bcast[:],
            )

        def drain(n_keep):
            while len(pending) > n_keep:
                kc, at_sl, qs, ww, yt, h, st, sp, action = pending.pop(0)
                nc.tensor.matmul(
                    yt[0:65, qs:qs + ww], v_sb[:, kc, h, 0:65],
                    at_sl[:, 0:ww],
                    start=st, stop=sp,
                    skip_group_check=True,
                )
                if action is not None:
                    action(yt, h)

        def attention_pass(h, yt, lo, hi, action, fill):
            j, po = h // 2, 64 * (h % 2)
            kT_h = kT_sb[po:po + 64, j, :]
            qT_h = qT_sb[po:po + 64, j, :]
            # shared groups (q width 512)
            kc = lo
            while kc < min(n_sh, hi):
                pa = pa_ps.tile([128, 4, 256], F32, tag="pa")
                if fill:
                    nc.tensor.matmul(   # HAM-warmth filler; overwritten by QK
                        pa[:, 0:2, :].rearrange("p a b -> p (a b)"),
                        id_b[:], qT_sb[:, 5, :], start=True, stop=True,
                    )
                for u in range(2):
                    nc.tensor.matmul(
                        pa[:, 2 * u:2 * u + 2, :].rearrange("p a b -> p (a b)"),
                        kT_h[:, (kc + u) * 128:(kc + u + 1) * 128],
                        qT_h[:, 0:512], start=True, stop=True,
                    )
                at = att_pool.tile([128, 2, 512], BF16, tag="at2")
                nc.scalar.activation(
                    out=at[:].rearrange("p a b -> p (a b)"),
                    in_=pa[:].rearrange("p a b -> p (a b)"),
                    func=mybir.ActivationFunctionType.Exp)
                for u in range(2):
                    if kc + u in (2 * sA, 2 * sA + 1):
                        nc.vector.tensor_mul(
                            at[:, u, 0:256], at[:, u, 0:256],
                            mask_t[:, kc + u - 2 * sA, :])
                for u in range(2):
                    pending.append((kc + u, at[:, u, :], 0, 512, yt, h,
                                    kc + u == lo, kc + u == hi - 1,
                                    action if kc + u == hi - 1 else None))
                kc += 2
                drain(PIPE_AV)
            # strip-B-only groups of up to 4 kt chunks (q width 256)
            while kc < hi:
                gsz = min(4, hi - kc)
                pa = pa_ps.tile([128, 4, 256], F32, tag="pa")
                if fill:
                    nc.tensor.matmul(   # HAM-warmth filler; overwritten by QK
                        pa[:, 0:2, :].rearrange("p a b -> p (a b)"),
                        id_b[:], qT_sb[:, 5, :], start=True, stop=True,
                    )
                for u in range(gsz):
                    nc.tensor.matmul(
                        pa[:, u, :], kT_h[:, (kc + u) * 128:(kc + u + 1) * 128],
                        qT_h[:, 256:512], start=True, stop=True,
                    )
                at = att_pool.tile([128, 4, 256], BF16, tag="at4")
                nc.scalar.activation(
                    out=at[:, 0:gsz, :].rearrange("p a b -> p (a b)"),
                    in_=pa[:, 0:gsz, :].rearrange("p a b -> p (a b)"),
                    func=mybir.ActivationFunctionType.Exp)
                for u in range(gsz):
                    if kc + u in (2 * sB, 2 * sB + 1):
                        nc.vector.tensor_mul(
                            at[:, u, :], at[:, u, :],
                            mask_t[:, kc + u - 2 * sB, :])
                for u in range(gsz):
                    pending.append((kc + u, at[:, u, :], 256, 256, yt, h,
                                    kc + u == lo, kc + u == hi - 1,
                                    action if kc + u == hi - 1 else None))
                kc += gsz
                drain(PIPE_AV)

        # ---- pass 1: chunks [0, C1) + deferred V GEMMs between heads ----
        vrest = list(range(4, NTK))
        for h in range(12):
            yt = yt_ps.tile([65, 512], F32, tag="yt")
            attention_pass(h, yt, 0, C1, evict_pass1, fill=False)
            for ti in vrest[(h * len(vrest)) // 12:((h + 1) * len(vrest)) // 12]:
                v_gemm_ti(ti)
        drain(0)
        # ---- pass 2: chunks [C1, n_all) + merge ----
        for h in range(12):
            yt = yt_ps.tile([65, 512], F32, tag="yt")
            attention_pass(h, yt, C1, n_all, finalize_head, fill=True)
        drain(0)
        s3.close()
        w12b.close()  # free wv/hT/pv

        # fc weights for stage 5 (DMA overlaps stage 4)
        wmlp = ctx.enter_context(tc.tile_pool(name="wmlp", bufs=1))
        wfc_sb = wmlp.tile([128, 6, DFF], BF16)
        nc.sync.dma_start(out=wfc_sb[:], in_=wfc_in[:].rearrange("(c k) n -> k c n", k=128))

        # ============ stage 4: c_proj + residual + LN2 + transpose ==========
        s4 = ExitStack()
        ln2_pool = s4.enter_context(tc.tile_pool(name="ln2", bufs=2))
        cp_ps = s4.enter_context(tc.tile_pool(name="cp_ps", bufs=3, space="PSUM"))
        tp2_ps = s4.enter_context(tc.tile_pool(name="tp2_ps", bufs=2, space="PSUM"))

        def cproj_m(m):
            pp = cp_ps.tile([128, 2, 512], F32, tag="cp")
            for half in range(2):
                for j in range(6):
                    nc.tensor.matmul(
                        pp[:, half, 0:384],
                        yT_sb[:, j, m * 128:(m + 1) * 128],
                        wcp_sb[:, j, half * 384:(half + 1) * 384],
                        start=(j == 0), stop=(j == 5),
                    )
            return pp

        def ln2_m(m, pp):
            x_own = ln2_pool.tile([128, C], F32, tag="xo")
            nc.sync.dma_start(out=x_own[:], in_=xo_in[m * 128:(m + 1) * 128, :])
            if WB:
                nc.vector.tensor_add(x_own[:], x_own[:], bcp_bc[:])
            nc.vector.tensor_add(
                x1_sb[:, m, :].rearrange("p (i n) -> p i n", i=2),
                pp[:, :, 0:384], x_own[:].rearrange("p (i n) -> p i n", i=2),
            )
            x1g = x1_sb[:, m, :].rearrange("p (g d) -> p g d", g=3)
            stats = ln2_pool.tile([128, 3, 6], F32, tag="st2")
            for g in range(3):
                nc.vector.bn_stats(out=stats[:, g, :], in_=x1g[:, g, :])
            mv = ln2_pool.tile([128, 2], F32, tag="mv2")
            nc.vector.bn_aggr(out=mv[:], in_=stats[:])
            rstd = ln2_pool.tile([128, 1], F32, tag="rstd2")
            nc.scalar.activation(
                out=rstd[:], in_=mv[:, 1:2],
                func=mybir.ActivationFunctionType.Sqrt,
                bias=eps_t[:], scale=1.0,
            )
            nc.vector.reciprocal(out=rstd[:], in_=rstd[:])
            h2 = ln2_pool.tile([128, C], BF16, tag="h2")
            nc.vector.tensor_scalar(
                out=h2[:], in0=x1_sb[:, m, :],
                scalar1=mv[:, 0:1], scalar2=rstd[:],
                op0=mybir.AluOpType.subtract, op1=mybir.AluOpType.mult,
            )
            return h2

        def tp2_m(m, h2):
            pt = tp2_ps.tile([128, 6, 128], BF16, tag="tp2")
            for c in range(6):
                nc.tensor.transpose(pt[:, c, :], h2[:, c * 128:(c + 1) * 128], id_b[:])
            nc.scalar.copy(
                h2T_sb[:, :, m * 128:(m + 1) * 128], pt[:],
            )

        # interleave so the DVE LN2 chain of block m runs under the PE
        # c_proj matmuls of later blocks
        pps, h2s = {}, {}
        pps[0] = cproj_m(0)
        pps[1] = cproj_m(1)
        h2s[0] = ln2_m(0, pps[0])
        pps[2] = cproj_m(2)
        h2s[1] = ln2_m(1, pps[1])
        pps[3] = cproj_m(3)
        tp2_m(0, h2s[0])
        h2s[2] = ln2_m(2, pps[2])
        tp2_m(1, h2s[1])
        h2s[3] = ln2_m(3, pps[3])
        tp2_m(2, h2s[2])
        tp2_m(3, h2s[3])
        s4.close()

        # ================== stage 5+6: fc+gelu, proj+store =================
        s56 = ExitStack()
        wpj_pool = s56.enter_context(tc.tile_pool(name="wpj", bufs=4))
        wpj_tiles = []
        for f in range(4):      # prefetch proj weights under the fc GEMMs
            wpj_t = wpj_pool.tile([128, C], BF16, tag="wpj")
            nc.sync.dma_start(out=wpj_t[:], in_=wpj_in[f * 128:(f + 1) * 128, :])
            wpj_tiles.append(wpj_t)

        s5 = ExitStack()
        pf_ps = s5.enter_context(tc.tile_pool(name="pf_ps", bufs=3, space="PSUM"))
        for f in range(24):
            pf = pf_ps.tile([128, 512], F32, tag="pf")
            for c in range(6):
                nc.tensor.matmul(
                    pf[:], wfc_sb[:, c, f * 128:(f + 1) * 128], h2T_sb[:, c, :],
                    start=(c == 0), stop=(c == 5),
                )
            nc.scalar.activation(
                out=gT_sb[:, f, :], in_=pf[:],
                func=mybir.ActivationFunctionType.Gelu_apprx_tanh,
                bias=(bfc_sb[:, f:f + 1] if WB else 0.0), scale=1.0,
            )
        s5.close()

        out_pool = s56.enter_context(tc.tile_pool(name="outp", bufs=3))
        pj_ps = s56.enter_context(tc.tile_pool(name="pj_ps", bufs=1, space="PSUM"))
        pj = []
        for i in range(8):
            pj_i = pj_ps.tile([128, 384], F32, tag=f"pj{i}")
            pj.append(pj_i)
        for f in range(24):
            if f < 4:
                wpj_t = wpj_tiles[f]
            else:
                wpj_t = wpj_pool.tile([128, C], BF16, tag="wpj")
                nc.sync.dma_start(out=wpj_t[:], in_=wpj_in[f * 128:(f + 1) * 128, :])
            for m in range(4):
                for half in range(2):
                    nc.tensor.matmul(
                        pj[m * 2 + half][:],
                        gT_sb[:, f, m * 128:(m + 1) * 128],
                        wpj_t[:, half * 384:(half + 1) * 384],
                        start=(f == 0), stop=(f == 23),
                    )
        for m in range(4):
            o_t = out_pool.tile([128, C], F32, tag="o")
            for half in range(2):
                nc.vector.tensor_add(
                    o_t[:, half * 384:(half + 1) * 384],
                    pj[m * 2 + half][:],
                    x1_sb[:, m, half * 384:(half + 1) * 384],
                )
                if WB:
                    nc.vector.tensor_add(
                        o_t[:, half * 384:(half + 1) * 384],
                        o_t[:, half * 384:(half + 1) * 384],
                        bpj_bc[:, half * 384:(half + 1) * 384],
                    )
                nc.sync.dma_start(
                    out=out_dram[m * 128:(m + 1) * 128, half * 384:(half + 1) * 384],
                    in_=o_t[:, half * 384:(half + 1) * 384])
        s56.close()


# ---------------------------------------------------------------------------
# Runner
# ---------------------------------------------------------------------------
def _make_runner(nc):
    partition_name = nc.partition_id_tensor.name if nc.partition_id_tensor else None
    in_names, out_names, out_avals, zero_outs = [], [], [], []
    for alloc in nc.m.functions[0].allocations:
        if not isinstance(alloc, mybir.MemoryLocationSet):
            continue
        name = alloc.memorylocations[0].name
        if alloc.kind == "ExternalInput":
            if name != partition_name:
                in_names.append(name)
        elif alloc.kind == "ExternalOutput":
            out_names.append(name)
            shape = tuple(alloc.tensor_shape)
            dtype = mybir.dt.np(alloc.dtype)
            out_avals.append(jax.core.ShapedArray(shape, dtype))
            zero_outs.append(np.zeros(shape, dtype))
    n_params = len(in_names)
    all_names = list(in_names) + list(out_names)
    if partition_name is not None:
        all_names.append(partition_name)

    def _body(*args):
        operands = list(args)
        if partition_name is not None:
            operands.append(partition_id_tensor())
        outs = _bass_exec_p.bind(
            *operands,
            out_avals=tuple(out_avals),
            in_names=tuple(all_names),
            out_names=tuple(out_names),
            lowering_input_output_aliases=(),
            sim_require_finite=True,
            sim_require_nnan=True,
            nc=nc,
        )
        return tuple(outs)

    donate = tuple(range(n_params, n_params + len(out_names)))
    jitted = jax.jit(_body, donate_argnums=donate, keep_unused=True)
    return jitted, in_names, out_names, zero_outs


@functools.lru_cache(maxsize=None)
def _get_runners(with_bias: bool):
    install_neuronx_cc_hook()
    runners = []
    for r in range(4):
        nc = build_rank_program(r, with_bias)
        runners.append(_make_runner(nc))
    return runners


def _prep_core_inputs(x, ln1_w, ln1_b, c_attn_w, c_attn_b, c_proj_w, c_proj_b,
                      ln2_w, ln2_b, fc_w, fc_b, proj_w, proj_b):
    """Fold LN affines into weights; split qkv; cast to bf16 on host."""
    import ml_dtypes
    f32 = np.float32
    bf16 = ml_dtypes.bfloat16
    wqkv = (ln1_w[:, None] * c_attn_w).astype(f32)
    bqkv = (c_attn_b + ln1_b @ c_attn_w).astype(f32)
    scale = f32(1.0 / np.sqrt(HD))
    biases = {
        "bq": np.ascontiguousarray(bqkv[0:C] * scale),
        "bk": np.ascontiguousarray(bqkv[C:2 * C]),
        "bv": np.ascontiguousarray(bqkv[2 * C:3 * C]),
        "bcp": np.ascontiguousarray(c_proj_b.astype(f32)),
        "bfc": np.ascontiguousarray((fc_b + ln2_b @ fc_w).astype(f32)),
        "bpj": np.ascontiguousarray(proj_b.astype(f32)),
    }
    with_bias = any(np.any(b != 0) for b in biases.values())
    shared = {
        "wq": np.ascontiguousarray((wqkv[:, 0:C] * scale).astype(bf16)),
        "wk": np.ascontiguousarray(wqkv[:, C:2 * C].astype(bf16)),
        "wv": np.ascontiguousarray(wqkv[:, 2 * C:3 * C].astype(bf16)),
        "wcp": np.ascontiguousarray(c_proj_w.astype(bf16)),
        "wfc": np.ascontiguousarray((ln2_w[:, None] * fc_w).astype(bf16)),
        "wpj": np.ascontiguousarray(proj_w.astype(bf16)),
    }
    if with_bias:
        shared.update(biases)
    return shared, with_bias


def _dispatch_all(inputs):
    """Dispatch the 8 per-core executions asynchronously; return futures."""
    import ml_dtypes
    shared, with_bias = _prep_core_inputs(
        **{k: np.asarray(v) for k, v in inputs.items()})
    runners = _get_runners(with_bias)
    devices = jax.devices()
    x = np.asarray(inputs["x"], dtype=np.float32)
    xb = x.astype(ml_dtypes.bfloat16)
    futs = []
    for c in range(8):
        b, r = c // 4, c % 4
        sA, sB = r, 7 - r
        jitted, in_names, out_names, zero_outs = runners[r]
        dev = devices[c]
        per_core = dict(shared)
        per_core["xb"] = np.ascontiguousarray(xb[b, :256 * (8 - r)])
        per_core["xo"] = np.ascontiguousarray(np.concatenate(
            [x[b, sA * 256:(sA + 1) * 256], x[b, sB * 256:(sB + 1) * 256]]))
        args = [jax.device_put(per_core[n], dev) for n in in_names]
        args += [jax.device_put(z, dev) for z in zero_outs]
        futs.append((c, out_names, jitted(*args)))
    return futs


def kernel(**inputs) -> np.ndarray:
    futs = _dispatch_all(inputs)
    out = np.empty((B, T, C), dtype=np.float32)
    for c, out_names, fut in futs:
        b, r = c // 4, c % 4
        res = np.asarray(fut[out_names.index("out")])
        out[b, 256 * r:256 * r + 256] = res[0:256]
        out[b, 256 * (7 - r):256 * (7 - r) + 256] = res[256:512]
    return out


# revision 30
# speedup vs baseline: 1.1786x; 1.1786x over previous
"""Trainium2 Bass kernel for a GPT-style transformer block (B=2, T=2048, C=768,
NH=12, HD=64, DFF=3072), distributed over 8 NeuronCores.

Sharding: token-data-parallel with zigzag strip assignment, zero collectives.
  - cores 0-3 process batch 0, cores 4-7 batch 1.
  - within a batch, rank r owns token strips r and 7-r (strips of 256 tokens).
  - each core redundantly computes K/V for tokens [0, 256*(8-r)) (its causal
    prefix), so no cross-core communication is needed at all.

All GEMM operands are bf16 (cast on host; fp32 PSUM accumulation), which
halves HBM traffic and SBUF footprint and removes all on-device weight casts.
One pass over all 12 heads; LN1+transpose is software-pipelined with the K/V
GEMMs per 512-token super-block. Attention softmax uses exp-without-max in
large batched ACT instructions with the normalization folded into the PSUM
eviction (ones-column trick).
"""

import sys
import types
import functools

sys.path.insert(0, "/opt/trn_rl_repo")

# ---- antenv.axon_hooks shim (missing module in this image) -----------------
if "antenv.axon_hooks" not in sys.modules:
    _hooks = types.ModuleType("antenv.axon_hooks")
    _hooks._hook = None
    _hooks.set_axon_ntff_profile_hook = lambda h: setattr(_hooks, "_hook", h)
    _hooks.get_axon_ntff_profile_hook = lambda: _hooks._hook
    sys.modules["antenv.axon_hooks"] = _hooks
    try:
        import antenv

        antenv.axon_hooks = _hooks
    except ImportError:
        pass

import numpy as np
import jax

import concourse.bass as bass
import concourse.mybir as mybir
import concourse.tile as tile
from concourse import bacc
from concourse.bass2jax import (
    _bass_exec_p,
    install_neuronx_cc_hook,
    partition_id_tensor,
)
from concourse.masks import make_identity

B, T, C = 2, 2048, 768
NH, HD, DFF = 12, 64, 64 * 48  # DFF = 3072
F32 = mybir.dt.float32
BF16 = mybir.dt.bfloat16
EPS = 1e-5


# ---------------------------------------------------------------------------
# Per-rank program builder
# ---------------------------------------------------------------------------
def build_rank_program(r: int, with_bias: bool):
    """Program for rank r (strips r and 7-r of one batch element)."""
    sA, sB = r, 7 - r
    NTK = 2 * (8 - r)          # kt tiles of 128 in the causal prefix
    T_kv = NTK * 128

    nc = bacc.Bacc("TRN2", target_bir_lowering=False, debug=False, num_devices=1)

    xb_in = nc.declare_dram_parameter("xb", [T_kv, C], BF16, isOutput=False)
    xo_in = nc.declare_dram_parameter("xo", [512, C], F32, isOutput=False)
    wq_in = nc.declare_dram_parameter("wq", [C, C], BF16, isOutput=False)
    wk_in = nc.declare_dram_parameter("wk", [C, C], BF16, isOutput=False)
    wv_in = nc.declare_dram_parameter("wv", [C, C], BF16, isOutput=False)
    wcp_in = nc.declare_dram_parameter("wcp", [C, C], BF16, isOutput=False)
    wfc_in = nc.declare_dram_parameter("wfc", [C, DFF], BF16, isOutput=False)
    wpj_in = nc.declare_dram_parameter("wpj", [DFF, C], BF16, isOutput=False)
    bias_ins = {}
    if with_bias:
        for nm, sz in (("bq", C), ("bk", C), ("bv", C), ("bcp", C),
                       ("bfc", DFF), ("bpj", C)):
            bias_ins[nm] = nc.declare_dram_parameter(nm, [sz], F32, isOutput=False)
    out_dram = nc.declare_dram_parameter("out", [512, C], F32, isOutput=True)

    with tile.TileContext(nc) as tc:
        _build_body(nc, tc, r, sA, sB, NTK, T_kv,
                    xb_in, xo_in, wq_in, wk_in, wv_in, wcp_in, wfc_in, wpj_in,
                    bias_ins, out_dram)
    nc.compile()
    return nc


def _build_body(nc, tc, r, sA, sB, NTK, T_kv,
                xb_in, xo_in, wq_in, wk_in, wv_in, wcp_in, wfc_in, wpj_in,
                bias_ins, out_dram):
    from contextlib import ExitStack

    WB = bool(bias_ins)
    n_sh = 2 * (sA + 1)            # kt chunks attended by both strips
    n_all = 2 * (sB + 1)           # kt chunks attended by strip B ( == NTK )
    assert n_all == NTK

    def strided2(base_ap, tbA, tbB, w):
        """Columns [tbA:tbA+w] and [tbB:tbB+w] of a [128, T] AP as [2, w]."""
        stride = base_ap.ap[-1][0]
        return bass.AP(
            tensor=base_ap.tensor,
            offset=base_ap[:, tbA:tbA + 1].offset,
            ap=[list(p) for p in base_ap.ap[:1]]
            + [[stride * (tbB - tbA), 2], [stride, w]],
        )

    with ExitStack() as ctx:
        # ------- constants -------
        const = ctx.enter_context(tc.tile_pool(name="const", bufs=1))
        id_f = const.tile([128, 128], F32)
        make_identity(nc, id_f[:])
        id_b = const.tile([128, 128], BF16)
        nc.vector.tensor_copy(id_b[:], id_f[:])
        eps_t = const.tile([128, 1], F32)
        nc.vector.memset(eps_t[:], EPS)
        # causal masks for the two in-strip kt chunk offsets: [128, 2, 256]
        mask_f = const.tile([128, 2, 256], F32)
        nc.vector.memset(mask_f[:], 1.0)
        for off in range(2):
            nc.gpsimd.affine_select(
                out=mask_f[:, off, :],
                in_=mask_f[:, off, :],
                compare_op=mybir.AluOpType.is_ge,
                fill=0.0,
                base=-128 * off,
                pattern=[[1, 256]],
                channel_multiplier=-1,
            )
        mask_t = const.tile([128, 2, 256], BF16)
        nc.vector.tensor_copy(mask_t[:], mask_f[:])

        if WB:
            bq_sb = const.tile([128, 6], F32)
            bk_sb = const.tile([128, 6], F32)
            for src, dst in ((bias_ins["bq"], bq_sb), (bias_ins["bk"], bk_sb)):
                nc.sync.dma_start(out=dst[:], in_=src[:].rearrange("(j p) -> p j", p=128))
            bfc_sb = const.tile([128, 24], F32)
            nc.sync.dma_start(out=bfc_sb[:], in_=bias_ins["bfc"][:].rearrange("(f p) -> p f", p=128))
            brow_f = const.tile([1, 3, C], F32)
            nc.sync.dma_start(out=brow_f[:, 0, :], in_=bias_ins["bv"][:][None, :])
            nc.sync.dma_start(out=brow_f[:, 1, :], in_=bias_ins["bcp"][:][None, :])
            nc.sync.dma_start(out=brow_f[:, 2, :], in_=bias_ins["bpj"][:][None, :])
            bias_bc = const.tile([128, 3, C], F32)
            nc.gpsimd.partition_broadcast(bias_bc[:], brow_f[:])
            bv_bc = bias_bc[:, 0, :]
            bcp_bc = bias_bc[:, 1, :]
            bpj_bc = bias_bc[:, 2, :]

        # ------- activation tensors spanning attention + MLP -------
        acts = ctx.enter_context(tc.tile_pool(name="acts", bufs=1))
        kT_sb = acts.tile([128, 6, T_kv], BF16)        # K^T, 12 heads
        v_sb = acts.tile([128, NTK, 12, 65], BF16)     # V natural + ones col
        qT_sb = acts.tile([128, 6, 512], BF16)         # Q^T for own strips
        yT_sb = acts.tile([128, 6, 512], BF16)         # attention out (normed)
        x1_sb = acts.tile([128, 4, C], F32)            # attn residual output
        h2T_sb = acts.tile([128, 6, 512], BF16)        # ln2 transposed
        gT_sb = acts.tile([128, 24, 512], BF16)        # gelu(fc) transposed
        wcp_sb = acts.tile([128, 6, C], BF16)          # c_proj weights

        nc.vector.memset(v_sb[:, :, :, 64:65], 1.0)

        # =========== stage 1+2: LN1 + transpose + K/V/Q GEMMs ==============
        s12 = ExitStack()
        w12 = s12.enter_context(tc.tile_pool(name="w12", bufs=1))
        wk_sb = w12.tile([128, 6, C], BF16)
        wv_sb = w12.tile([128, 6, C], BF16)
        wq_sb = w12.tile([128, 6, C], BF16)
        hT_sb = w12.tile([128, 6, T_kv], BF16)
        # x tiles for the first super-block are needed immediately; weight
        # DMAs go behind them in the queue.
        xpre_pool = s12.enter_context(tc.tile_pool(name="xpre", bufs=1))
        xpre = xpre_pool.tile([128, 4, C], BF16)
        for tt in range(min(4, T_kv // 128)):
            nc.sync.dma_start(out=xpre[:, tt, :],
                              in_=xb_in[tt * 128:(tt + 1) * 128, :])
        for src, dst in ((wk_in, wk_sb), (wv_in, wv_sb), (wq_in, wq_sb),
                         (wcp_in, wcp_sb)):
            nc.sync.dma_start(out=dst[:], in_=src[:].rearrange("(c k) n -> k c n", k=128))

        ln_pool = s12.enter_context(tc.tile_pool(name="ln", bufs=3))
        gemm_ps = s12.enter_context(tc.tile_pool(name="gemm_ps", bufs=2, space="PSUM"))
        pv_ps = s12.enter_context(tc.tile_pool(name="pv_ps", bufs=1, space="PSUM"))
        tp_ps = s12.enter_context(tc.tile_pool(name="tp_ps", bufs=2, space="PSUM"))

        # super-blocks of up to 512 tokens
        sblocks = []
        t0 = 0
        while t0 < T_kv:
            w = min(512, T_kv - t0)
            sblocks.append((t0, w))
            t0 += w

        # ---- LN1 + transpose for all tiles (DVE pipelines under PE) ----
        for ti in range(T_kv // 128):
            if ti < 4:
                x_t = xpre[:, ti, :]
            else:
                x_t = ln_pool.tile([128, C], BF16, tag="x")
                nc.sync.dma_start(
                    out=x_t[:],
                    in_=xb_in[ti * 128:(ti + 1) * 128, :])
            xg = x_t.rearrange("p (g d) -> p g d", g=3)
            stats = ln_pool.tile([128, 3, 6], F32, tag="st")
            for g in range(3):
                nc.vector.bn_stats(out=stats[:, g, :], in_=xg[:, g, :])
            mv = ln_pool.tile([128, 2], F32, tag="mv")
            nc.vector.bn_aggr(out=mv[:], in_=stats[:])
            rstd = ln_pool.tile([128, 1], F32, tag="rstd")
            nc.scalar.activation(
                out=rstd[:], in_=mv[:, 1:2],
                func=mybir.ActivationFunctionType.Sqrt,
                bias=eps_t[:], scale=1.0,
            )
            nc.vector.reciprocal(out=rstd[:], in_=rstd[:])
            h_t = ln_pool.tile([128, C], BF16, tag="h")
            nc.vector.tensor_scalar(
                out=h_t[:], in0=x_t[:],
                scalar1=mv[:, 0:1], scalar2=rstd[:],
                op0=mybir.AluOpType.subtract, op1=mybir.AluOpType.mult,
            )
            pt = tp_ps.tile([128, 6, 128], BF16, tag="tp")
            for c in range(6):
                nc.tensor.transpose(pt[:, c, :], h_t[:, c * 128:(c + 1) * 128], id_b[:])
            # evict all 6 transposed chunks in one ACT copy (DVE is busy)
            nc.scalar.copy(
                hT_sb[:, :, ti * 128:(ti + 1) * 128], pt[:],
            )
        # ---- K GEMM: stationary wk chunk reused across all super-blocks ----
        for jj in range(6):
            pks = []
            for bi in range(len(sblocks)):
                pk_i = gemm_ps.tile([128, 512], F32, tag=f"pk{bi}", bufs=1)
                pks.append(pk_i)
            for c in range(6):
                for bi, (tb, bw) in enumerate(sblocks):
                    nc.tensor.matmul(
                        pks[bi][:, 0:bw], wk_sb[:, c, jj * 128:(jj + 1) * 128],
                        hT_sb[:, c, tb:tb + bw],
                        start=(c == 0), stop=(c == 5),
                    )
            for bi, (tb, bw) in enumerate(sblocks):
                if WB:
                    nc.vector.tensor_scalar(
                        out=kT_sb[:, jj, tb:tb + bw], in0=pks[bi][:, 0:bw],
                        scalar1=bk_sb[:, jj:jj + 1], scalar2=None,
                        op0=mybir.AluOpType.add,
                    )
                else:
                    nc.vector.tensor_copy(kT_sb[:, jj, tb:tb + bw], pks[bi][:, 0:bw])
        # ---- V GEMM: stationary hT chunk reused across both halves ----
        for ti in range(T_kv // 128):
            pv0 = pv_ps.tile([128, 384], F32, tag="pv0")
            pv1 = pv_ps.tile([128, 384], F32, tag="pv1")
            for c in range(6):
                for half, pv in ((0, pv0), (1, pv1)):
                    nc.tensor.matmul(
                        pv[:], hT_sb[:, c, ti * 128:(ti + 1) * 128],
                        wv_sb[:, c, half * 384:(half + 1) * 384],
                        start=(c == 0), stop=(c == 5),
                    )
            for half, pv in ((0, pv0), (1, pv1)):
                h0 = half * 6
                vdst = v_sb[:, ti, h0:h0 + 6, 0:64]
                if WB:
                    nc.vector.tensor_tensor(
                        out=vdst,
                        in0=pv[:].rearrange("p (h d) -> p h d", d=64),
                        in1=bv_bc[:, h0 * 64:(h0 + 6) * 64].rearrange(
                            "p (h d) -> p h d", d=64),
                        op=mybir.AluOpType.add,
                    )
                else:
                    nc.vector.tensor_copy(
                        vdst, pv[:].rearrange("p (h d) -> p h d", d=64))

        # Q GEMM for own strips (both strips in one N=512 matmul)
        tbA, tbB = sA * 256, sB * 256
        for jj in range(6):
            pq = gemm_ps.tile([128, 512], F32, tag="pk0", bufs=1)
            for c in range(6):
                rhs = strided2(hT_sb[:, c, :], tbA, tbB, 256)
                nc.tensor.matmul(
                    pq[:], wq_sb[:, c, jj * 128:(jj + 1) * 128], rhs,
                    start=(c == 0), stop=(c == 5),
                )
            if WB:
                nc.vector.tensor_scalar(
                    out=qT_sb[:, jj, :], in0=pq[:],
                    scalar1=bq_sb[:, jj:jj + 1], scalar2=None,
                    op0=mybir.AluOpType.add,
                )
            else:
                nc.vector.tensor_copy(qT_sb[:, jj, :], pq[:])

        s12.close()  # free wk/wv/wq/hT SBUF

        # prefetch fc weights during attention (into the space s12 freed)
        wmlp = ctx.enter_context(tc.tile_pool(name="wmlp", bufs=1))
        wfc_sb = wmlp.tile([128, 6, DFF], BF16)
        nc.sync.dma_start(out=wfc_sb[:], in_=wfc_in[:].rearrange("(c k) n -> k c n", k=128))

        # ======================= stage 3: attention ========================
        s3 = ExitStack()
        att_pool = s3.enter_context(tc.tile_pool(name="att", bufs=4))
        nrm_pool = s3.enter_context(tc.tile_pool(name="nrm", bufs=2))
        pa_ps = s3.enter_context(tc.tile_pool(name="pa_ps", bufs=3, space="PSUM"))
        yt_ps = s3.enter_context(tc.tile_pool(name="yt_ps", bufs=2, space="PSUM"))

        # Cross-head software pipeline: AV matmuls run PIPE_AV chunks behind
        # the QK/exp frontier, so the PE always has independent QK work (from
        # the next head) queued ahead of exp-dependent AVs.  A small dummy
        # matmul per group tops up PE occupancy so the HAM clock gate stays
        # at full rate through this ACT-bound phase.
        PIPE_AV = 5
        pending = []   # (kc, at_slice, qs, ww, yt_handle, h, is_last)

        def finalize_head(yt, h):
            j, po = h // 2, 64 * (h % 2)
            sume = nrm_pool.tile([1, 512], F32, tag="sume")
            nc.vector.tensor_copy(sume[:], yt[64:65, :])
            bcast = nrm_pool.tile([64, 512], F32, tag="bcast")
            nc.gpsimd.partition_broadcast(bcast[:], sume[:])
            nc.vector.reciprocal_approx_fast(out=bcast[:], in_=bcast[:])
            nc.vector.tensor_mul(
                yT_sb[po:po + 64, j, :], yt[0:64, :], bcast[:],
            )

        def drain(n_keep):
            while len(pending) > n_keep:
                kc, at_sl, qs, ww, yt, h, last = pending.pop(0)
                nc.tensor.matmul(
                    yt[0:65, qs:qs + ww], v_sb[:, kc, h, 0:65],
                    at_sl[:, 0:ww],
                    start=(kc == 0), stop=(kc == n_all - 1),
                    skip_group_check=True,
                )
                if last:
                    finalize_head(yt, h)

        for h in range(12):
            j, po = h // 2, 64 * (h % 2)
            kT_h = kT_sb[po:po + 64, j, :]
            qT_h = qT_sb[po:po + 64, j, :]
            yt = yt_ps.tile([65, 512], F32, tag="yt")

            # shared groups: pairs of kt chunks seen by both strips (q width 512)
            for g in range(n_sh // 2):
                kc0 = 2 * g
                pa = pa_ps.tile([128, 4, 256], F32, tag="pa")
                nc.tensor.matmul(   # HAM-warmth filler; overwritten by QK
                    pa[:, 0:2, :].rearrange("p a b -> p (a b)"),
                    id_b[:], qT_sb[:, 5, :], start=True, stop=True,
                )
                for u in range(2):
                    nc.tensor.matmul(
                        pa[:, 2 * u:2 * u + 2, :].rearrange("p a b -> p (a b)"),
                        kT_h[:, (kc0 + u) * 128:(kc0 + u + 1) * 128],
                        qT_h[:, 0:512], start=True, stop=True,
                    )
                at = att_pool.tile([128, 2, 512], BF16, tag="at2")
                nc.scalar.activation(
                    out=at[:].rearrange("p a b -> p (a b)"),
                    in_=pa[:].rearrange("p a b -> p (a b)"),
                    func=mybir.ActivationFunctionType.Exp)
                for u in range(2):
                    kc = kc0 + u
                    if kc in (2 * sA, 2 * sA + 1):
                        nc.vector.tensor_mul(
                            at[:, u, 0:256], at[:, u, 0:256],
                            mask_t[:, kc - 2 * sA, :])
                pending.append((kc0, at[:, 0, :], 0, 512, yt, h, False))
                pending.append((kc0 + 1, at[:, 1, :], 0, 512, yt, h,
                                kc0 + 1 == n_all - 1))
                drain(PIPE_AV)

            # strip-B-only groups of up to 4 kt chunks (q width 256)
            kc = n_sh
            while kc < n_all:
                gsz = min(4, n_all - kc)
                pa = pa_ps.tile([128, 4, 256], F32, tag="pa")
                nc.tensor.matmul(   # HAM-warmth filler; overwritten by QK
                    pa[:, 0:2, :].rearrange("p a b -> p (a b)"),
                    id_b[:], qT_sb[:, 5, :], start=True, stop=True,
                )
                for u in range(gsz):
                    nc.tensor.matmul(
                        pa[:, u, :], kT_h[:, (kc + u) * 128:(kc + u + 1) * 128],
                        qT_h[:, 256:512], start=True, stop=True,
                    )
                at = att_pool.tile([128, 4, 256], BF16, tag="at4")
                nc.scalar.activation(
                    out=at[:, 0:gsz, :].rearrange("p a b -> p (a b)"),
                    in_=pa[:, 0:gsz, :].rearrange("p a b -> p (a b)"),
                    func=mybir.ActivationFunctionType.Exp)
                for u in range(gsz):
                    if kc + u in (2 * sB, 2 * sB + 1):
                        nc.vector.tensor_mul(
                            at[:, u, :], at[:, u, :],
                            mask_t[:, kc + u - 2 * sB, :])
                for u in range(gsz):
                    pending.append((kc + u, at[:, u, :], 256, 256, yt, h,
                                    kc + u == n_all - 1))
                kc += gsz
                drain(PIPE_AV)
        drain(0)
        s3.close()

        # ============ stage 4: c_proj + residual + LN2 + transpose ==========
        s4 = ExitStack()
        ln2_pool = s4.enter_context(tc.tile_pool(name="ln2", bufs=2))
        cp_ps = s4.enter_context(tc.tile_pool(name="cp_ps", bufs=3, space="PSUM"))
        tp2_ps = s4.enter_context(tc.tile_pool(name="tp2_ps", bufs=2, space="PSUM"))

        def cproj_m(m):
            pp = cp_ps.tile([128, 2, 512], F32, tag="cp")
            for half in range(2):
                for j in range(6):
                    nc.tensor.matmul(
                        pp[:, half, 0:384],
                        yT_sb[:, j, m * 128:(m + 1) * 128],
                        wcp_sb[:, j, half * 384:(half + 1) * 384],
                        start=(j == 0), stop=(j == 5),
                    )
            return pp

        def ln2_m(m, pp):
            x_own = ln2_pool.tile([128, C], F32, tag="xo")
            nc.sync.dma_start(out=x_own[:], in_=xo_in[m * 128:(m + 1) * 128, :])
            if WB:
                nc.vector.tensor_add(x_own[:], x_own[:], bcp_bc[:])
            nc.vector.tensor_add(
                x1_sb[:, m, :].rearrange("p (i n) -> p i n", i=2),
                pp[:, :, 0:384], x_own[:].rearrange("p (i n) -> p i n", i=2),
            )
            x1g = x1_sb[:, m, :].rearrange("p (g d) -> p g d", g=3)
            stats = ln2_pool.tile([128, 3, 6], F32, tag="st2")
            for g in range(3):
                nc.vector.bn_stats(out=stats[:, g, :], in_=x1g[:, g, :])
            mv = ln2_pool.tile([128, 2], F32, tag="mv2")
            nc.vector.bn_aggr(out=mv[:], in_=stats[:])
            rstd = ln2_pool.tile([128, 1], F32, tag="rstd2")
            nc.scalar.activation(
                out=rstd[:], in_=mv[:, 1:2],
                func=mybir.ActivationFunctionType.Sqrt,
                bias=eps_t[:], scale=1.0,
            )
            nc.vector.reciprocal(out=rstd[:], in_=rstd[:])
            h2 = ln2_pool.tile([128, C], BF16, tag="h2")
            nc.vector.tensor_scalar(
                out=h2[:], in0=x1_sb[:, m, :],
                scalar1=mv[:, 0:1], scalar2=rstd[:],
                op0=mybir.AluOpType.subtract, op1=mybir.AluOpType.mult,
            )
            return h2

        def tp2_m(m, h2):
            pt = tp2_ps.tile([128, 6, 128], BF16, tag="tp2")
            for c in range(6):
                nc.tensor.transpose(pt[:, c, :], h2[:, c * 128:(c + 1) * 128], id_b[:])
            nc.scalar.copy(
                h2T_sb[:, :, m * 128:(m + 1) * 128], pt[:],
            )

        # interleave so the DVE LN2 chain of block m runs under the PE
        # c_proj matmuls of later blocks
        pps, h2s = {}, {}
        pps[0] = cproj_m(0)
        pps[1] = cproj_m(1)
        h2s[0] = ln2_m(0, pps[0])
        pps[2] = cproj_m(2)
        h2s[1] = ln2_m(1, pps[1])
        pps[3] = cproj_m(3)
        tp2_m(0, h2s[0])
        h2s[2] = ln2_m(2, pps[2])
        tp2_m(1, h2s[1])
        h2s[3] = ln2_m(3, pps[3])
        tp2_m(2, h2s[2])
        tp2_m(3, h2s[3])
        s4.close()

        # ================== stage 5+6: fc+gelu, proj+store =================
        s56 = ExitStack()
        wpj_pool = s56.enter_context(tc.tile_pool(name="wpj", bufs=4))
        wpj_tiles = []
        for f in range(4):      # prefetch proj weights under the fc GEMMs
            wpj_t = wpj_pool.tile([128, C], BF16, tag="wpj")
            nc.sync.dma_start(out=wpj_t[:], in_=wpj_in[f * 128:(f + 1) * 128, :])
            wpj_tiles.append(wpj_t)

        s5 = ExitStack()
        pf_ps = s5.enter_context(tc.tile_pool(name="pf_ps", bufs=3, space="PSUM"))
        for f in range(24):
            pf = pf_ps.tile([128, 512], F32, tag="pf")
            for c in range(6):
                nc.tensor.matmul(
                    pf[:], wfc_sb[:, c, f * 128:(f + 1) * 128], h2T_sb[:, c, :],
                    start=(c == 0), stop=(c == 5),
                )
            nc.scalar.activation(
                out=gT_sb[:, f, :], in_=pf[:],
                func=mybir.ActivationFunctionType.Gelu_apprx_tanh,
                bias=(bfc_sb[:, f:f + 1] if WB else 0.0), scale=1.0,
            )
        s5.close()

        out_pool = s56.enter_context(tc.tile_pool(name="outp", bufs=3))
        pj_ps = s56.enter_context(tc.tile_pool(name="pj_ps", bufs=1, space="PSUM"))
        pj = []
        for i in range(8):
            pj_i = pj_ps.tile([128, 384], F32, tag=f"pj{i}")
            pj.append(pj_i)
        for f in range(24):
            if f < 4:
                wpj_t = wpj_tiles[f]
            else:
                wpj_t = wpj_pool.tile([128, C], BF16, tag="wpj")
                nc.sync.dma_start(out=wpj_t[:], in_=wpj_in[f * 128:(f + 1) * 128, :])
            for m in range(4):
                for half in range(2):
                    nc.tensor.matmul(
                        pj[m * 2 + half][:],
                        gT_sb[:, f, m * 128:(m + 1) * 128],
                        wpj_t[:, half * 384:(half + 1) * 384],
                        start=(f == 0), stop=(f == 23),
                    )
        for m in range(4):
            o_t = out_pool.tile([128, C], F32, tag="o")
            for half in range(2):
                nc.vector.tensor_add(
                    o_t[:, half * 384:(half + 1) * 384],
                    pj[m * 2 + half][:],
                    x1_sb[:, m, half * 384:(half + 1) * 384],
                )
                if WB:
                    nc.vector.tensor_add(
                        o_t[:, half * 384:(half + 1) * 384],
                        o_t[:, half * 384:(half + 1) * 384],
                        bpj_bc[:, half * 384:(half + 1) * 384],
                    )
                nc.sync.dma_start(
                    out=out_dram[m * 128:(m + 1) * 128, half * 384:(half + 1) * 384],
                    in_=o_t[:, half * 384:(half + 1) * 384])
        s56.close()


# ---------------------------------------------------------------------------
# Runner
# ---------------------------------------------------------------------------
def _make_runner(nc):
    partition_name = nc.partition_id_tensor.name if nc.partition_id_tensor else None
    in_names, out_names, out_avals, zero_outs = [], [], [], []
    for alloc in nc.m.functions[0].allocations:
        if not isinstance(alloc, mybir.MemoryLocationSet):
            continue
        name = alloc.memorylocations[0].name
        if alloc.kind == "ExternalInput":
            if name != partition_name:
                in_names.append(name)
        elif alloc.kind == "ExternalOutput":
            out_names.append(name)
            shape = tuple(alloc.tensor_shape)
            dtype = mybir.dt.np(alloc.dtype)
            out_avals.append(jax.core.ShapedArray(shape, dtype))
            zero_outs.append(np.zeros(shape, dtype))
    n_params = len(in_names)
    all_names = list(in_names) + list(out_names)
    if partition_name is not None:
        all_names.append(partition_name)

    def _body(*args):
        operands = list(args)
        if partition_name is not None:
            operands.append(partition_id_tensor())
        outs = _bass_exec_p.bind(
            *operands,
            out_avals=tuple(out_avals),
            in_names=tuple(all_names),
            out_names=tuple(out_names),
            lowering_input_output_aliases=(),
            sim_require_finite=True,
            sim_require_nnan=True,
            nc=nc,
        )
        return tuple(outs)

    donate = tuple(range(n_params, n_params + len(out_names)))
    jitted = jax.jit(_body, donate_argnums=donate, keep_unused=True)
    return jitted, in_names, out_names, zero_outs


@functools.lru_cache(maxsize=None)
def _get_runners(with_bias: bool):
    install_neuronx_cc_hook()
    runners = []
    for r in range(4):
        nc = build_rank_program(r, with_bias)
        runners.append(_make_runner(nc))
    return runners


def _prep_core_inputs(x, ln1_w, ln1_b, c_attn_w, c_attn_b, c_proj_w, c_proj_b,
                      ln2_w, ln2_b, fc_w, fc_b, proj_w, proj_b):
    """Fold LN affines into weights; split qkv; cast to bf16 on host."""
    import ml_dtypes
    f32 = np.float32
    bf16 = ml_dtypes.bfloat16
    wqkv = (ln1_w[:, None] * c_attn_w).astype(f32)
    bqkv = (c_attn_b + ln1_b @ c_attn_w).astype(f32)
    scale = f32(1.0 / np.sqrt(HD))
    biases = {
        "bq": np.ascontiguousarray(bqkv[0:C] * scale),
        "bk": np.ascontiguousarray(bqkv[C:2 * C]),
        "bv": np.ascontiguousarray(bqkv[2 * C:3 * C]),
        "bcp": np.ascontiguousarray(c_proj_b.astype(f32)),
        "bfc": np.ascontiguousarray((fc_b + ln2_b @ fc_w).astype(f32)),
        "bpj": np.ascontiguousarray(proj_b.astype(f32)),
    }
    with_bias = any(np.any(b != 0) for b in biases.values())
    shared = {
        "wq": np.ascontiguousarray((wqkv[:, 0:C] * scale).astype(bf16)),
        "wk": np.ascontiguousarray(wqkv[:, C:2 * C].astype(bf16)),
        "wv": np.ascontiguousarray(wqkv[:, 2 * C:3 * C].astype(bf16)),
        "wcp": np.ascontiguousarray(c_proj_w.astype(bf16)),
        "wfc": np.ascontiguousarray((ln2_w[:, None] * fc_w).astype(bf16)),
        "wpj": np.ascontiguousarray(proj_w.astype(bf16)),
    }
    if with_bias:
        shared.update(biases)
    return shared, with_bias


def _dispatch_all(inputs):
    """Dispatch the 8 per-core executions asynchronously; return futures."""
    import ml_dtypes
    shared, with_bias = _prep_core_inputs(
        **{k: np.asarray(v) for k, v in inputs.items()})
    runners = _get_runners(with_bias)
    devices = jax.devices()
    x = np.asarray(inputs["x"], dtype=np.float32)
    xb = x.astype(ml_dtypes.bfloat16)
    futs = []
    for c in range(8):
        b, r = c // 4, c % 4
        sA, sB = r, 7 - r
        jitted, in_names, out_names, zero_outs = runners[r]
        dev = devices[c]
        per_core = dict(shared)
        per_core["xb"] = np.ascontiguousarray(xb[b, :256 * (8 - r)])
        per_core["xo"] = np.ascontiguousarray(np.concatenate(
            [x[b, sA * 256:(sA + 1) * 256], x[b, sB * 256:(sB + 1) * 256]]))
        args = [jax.device_put(per_core[n], dev) for n in in_names]
        args += [jax.device_put(z, dev) for z in zero_outs]
        futs.append((c, out_names, jitted(*args)))
    return futs


def kernel(**inputs) -> np.ndarray:
    futs = _dispatch_all(inputs)
    out = np.empty((B, T, C), dtype=np.float32)
    for c, out_names, fut in futs:
        b, r = c // 4, c % 4
        res = np.asarray(fut[out_names.index("out")])
        out[b, 256 * r:256 * r + 256] = res[0:256]
        out[b, 256 * (7 - r):256 * (7 - r) + 256] = res[256:512]
    return out


# revision 32
# speedup vs baseline: 1.2299x; 1.0435x over previous
"""Trainium2 Bass kernel for a GPT-style transformer block (B=2, T=2048, C=768,
NH=12, HD=64, DFF=3072), distributed over 8 NeuronCores.

Sharding: token-data-parallel with zigzag strip assignment, zero collectives.
  - cores 0-3 process batch 0, cores 4-7 batch 1.
  - within a batch, rank r owns token strips r and 7-r (strips of 256 tokens).
  - each core redundantly computes K/V for tokens [0, 256*(8-r)) (its causal
    prefix), so no cross-core communication is needed at all.

All GEMM operands are bf16 (cast on host; fp32 PSUM accumulation), which
halves HBM traffic and SBUF footprint and removes all on-device weight casts.
One pass over all 12 heads; LN1+transpose is software-pipelined with the K/V
GEMMs per 512-token super-block. Attention softmax uses exp-without-max in
large batched ACT instructions with the normalization folded into the PSUM
eviction (ones-column trick).
"""

import sys
import types
import functools

sys.path.insert(0, "/opt/trn_rl_repo")

# ---- antenv.axon_hooks shim (missing module in this image) -----------------
if "antenv.axon_hooks" not in sys.modules:
    _hooks = types.ModuleType("antenv.axon_hooks")
    _hooks._hook = None
    _hooks.set_axon_ntff_profile_hook = lambda h: setattr(_hooks, "_hook", h)
    _hooks.get_axon_ntff_profile_hook = lambda: _hooks._hook
    sys.modules["antenv.axon_hooks"] = _hooks
    try:
        import antenv

        antenv.axon_hooks = _hooks
    except ImportError:
        pass

import numpy as np
import jax

import concourse.bass as bass
import concourse.mybir as mybir
import concourse.tile as tile
from concourse import bacc
from concourse.bass2jax import (
    _bass_exec_p,
    install_neuronx_cc_hook,
    partition_id_tensor,
)
from concourse.masks import make_identity

B, T, C = 2, 2048, 768
NH, HD, DFF = 12, 64, 64 * 48  # DFF = 3072
F32 = mybir.dt.float32
BF16 = mybir.dt.bfloat16
EPS = 1e-5


# ---------------------------------------------------------------------------
# Per-rank program builder
# ---------------------------------------------------------------------------
def build_rank_program(r: int, with_bias: bool):
    """Program for rank r (strips r and 7-r of one batch element)."""
    sA, sB = r, 7 - r
    NTK = 2 * (8 - r)          # kt tiles of 128 in the causal prefix
    T_kv = NTK * 128

    nc = bacc.Bacc("TRN2", target_bir_lowering=False, debug=False, num_devices=1)

    xb_in = nc.declare_dram_parameter("xb", [T_kv, C], BF16, isOutput=False)
    xo_in = nc.declare_dram_parameter("xo", [512, C], F32, isOutput=False)
    wq_in = nc.declare_dram_parameter("wq", [C, C], BF16, isOutput=False)
    wk_in = nc.declare_dram_parameter("wk", [C, C], BF16, isOutput=False)
    wv_in = nc.declare_dram_parameter("wv", [C, C], BF16, isOutput=False)
    wcp_in = nc.declare_dram_parameter("wcp", [C, C], BF16, isOutput=False)
    wfc_in = nc.declare_dram_parameter("wfc", [C, DFF], BF16, isOutput=False)
    wpj_in = nc.declare_dram_parameter("wpj", [DFF, C], BF16, isOutput=False)
    bias_ins = {}
    if with_bias:
        for nm, sz in (("bq", C), ("bk", C), ("bv", C), ("bcp", C),
                       ("bfc", DFF), ("bpj", C)):
            bias_ins[nm] = nc.declare_dram_parameter(nm, [sz], F32, isOutput=False)
    out_dram = nc.declare_dram_parameter("out", [512, C], F32, isOutput=True)

    with tile.TileContext(nc) as tc:
        _build_body(nc, tc, r, sA, sB, NTK, T_kv,
                    xb_in, xo_in, wq_in, wk_in, wv_in, wcp_in, wfc_in, wpj_in,
                    bias_ins, out_dram)
    nc.compile()
    return nc


def _build_body(nc, tc, r, sA, sB, NTK, T_kv,
                xb_in, xo_in, wq_in, wk_in, wv_in, wcp_in, wfc_in, wpj_in,
                bias_ins, out_dram):
    from contextlib import ExitStack

    WB = bool(bias_ins)
    n_sh = 2 * (sA + 1)            # kt chunks attended by both strips
    n_all = 2 * (sB + 1)           # kt chunks attended by strip B ( == NTK )
    assert n_all == NTK

    def strided2(base_ap, tbA, tbB, w):
        """Columns [tbA:tbA+w] and [tbB:tbB+w] of a [128, T] AP as [2, w]."""
        stride = base_ap.ap[-1][0]
        return bass.AP(
            tensor=base_ap.tensor,
            offset=base_ap[:, tbA:tbA + 1].offset,
            ap=[list(p) for p in base_ap.ap[:1]]
            + [[stride * (tbB - tbA), 2], [stride, w]],
        )

    with ExitStack() as ctx:
        # ------- constants -------
        const = ctx.enter_context(tc.tile_pool(name="const", bufs=1))
        id_f = const.tile([128, 128], F32)
        make_identity(nc, id_f[:])
        id_b = const.tile([128, 128], BF16)
        nc.vector.tensor_copy(id_b[:], id_f[:])
        eps_t = const.tile([128, 1], F32)
        nc.vector.memset(eps_t[:], EPS)
        # causal masks for the two in-strip kt chunk offsets: [128, 2, 256]
        mask_f = const.tile([128, 2, 256], F32)
        nc.vector.memset(mask_f[:], 1.0)
        for off in range(2):
            nc.gpsimd.affine_select(
                out=mask_f[:, off, :],
                in_=mask_f[:, off, :],
                compare_op=mybir.AluOpType.is_ge,
                fill=0.0,
                base=-128 * off,
                pattern=[[1, 256]],
                channel_multiplier=-1,
            )
        mask_t = const.tile([128, 2, 256], BF16)
        nc.vector.tensor_copy(mask_t[:], mask_f[:])

        if WB:
            bq_sb = const.tile([128, 6], F32)
            bk_sb = const.tile([128, 6], F32)
            for src, dst in ((bias_ins["bq"], bq_sb), (bias_ins["bk"], bk_sb)):
                nc.sync.dma_start(out=dst[:], in_=src[:].rearrange("(j p) -> p j", p=128))
            bfc_sb = const.tile([128, 24], F32)
            nc.sync.dma_start(out=bfc_sb[:], in_=bias_ins["bfc"][:].rearrange("(f p) -> p f", p=128))
            brow_f = const.tile([1, 3, C], F32)
            nc.sync.dma_start(out=brow_f[:, 0, :], in_=bias_ins["bv"][:][None, :])
            nc.sync.dma_start(out=brow_f[:, 1, :], in_=bias_ins["bcp"][:][None, :])
            nc.sync.dma_start(out=brow_f[:, 2, :], in_=bias_ins["bpj"][:][None, :])
            bias_bc = const.tile([128, 3, C], F32)
            nc.gpsimd.partition_broadcast(bias_bc[:], brow_f[:])
            bv_bc = bias_bc[:, 0, :]
            bcp_bc = bias_bc[:, 1, :]
            bpj_bc = bias_bc[:, 2, :]

        # ------- activation tensors spanning attention + MLP -------
        acts = ctx.enter_context(tc.tile_pool(name="acts", bufs=1))
        kT_sb = acts.tile([128, 6, T_kv], BF16)        # K^T, 12 heads
        v_sb = acts.tile([128, NTK, 12, 65], BF16)     # V natural + ones col
        qT_sb = acts.tile([128, 6, 512], BF16)         # Q^T for own strips
        yT_sb = acts.tile([128, 6, 512], BF16)         # attention out (normed)
        x1_sb = acts.tile([128, 4, C], F32)            # attn residual output
        h2T_sb = acts.tile([128, 6, 512], BF16)        # ln2 transposed
        gT_sb = acts.tile([128, 24, 512], BF16)        # gelu(fc) transposed
        wcp_sb = acts.tile([128, 6, C], BF16)          # c_proj weights

        nc.vector.memset(v_sb[:, :, :, 64:65], 1.0)

        # =========== stage 1+2: LN1 + transpose + K/V/Q GEMMs ==============
        s12 = ExitStack()
        w12 = s12.enter_context(tc.tile_pool(name="w12", bufs=1))
        wk_sb = w12.tile([128, 6, C], BF16)
        wv_sb = w12.tile([128, 6, C], BF16)
        wq_sb = w12.tile([128, 6, C], BF16)
        hT_sb = w12.tile([128, 6, T_kv], BF16)
        # x tiles for the first super-block are needed immediately; weight
        # DMAs go behind them in the queue.
        xpre_pool = s12.enter_context(tc.tile_pool(name="xpre", bufs=1))
        xpre = xpre_pool.tile([128, 4, C], BF16)
        for tt in range(min(4, T_kv // 128)):
            nc.sync.dma_start(out=xpre[:, tt, :],
                              in_=xb_in[tt * 128:(tt + 1) * 128, :])
        for src, dst in ((wk_in, wk_sb), (wv_in, wv_sb), (wq_in, wq_sb),
                         (wcp_in, wcp_sb)):
            nc.sync.dma_start(out=dst[:], in_=src[:].rearrange("(c k) n -> k c n", k=128))

        ln_pool = s12.enter_context(tc.tile_pool(name="ln", bufs=4))
        gemm_ps = s12.enter_context(tc.tile_pool(name="gemm_ps", bufs=3, space="PSUM"))
        pv_ps = s12.enter_context(tc.tile_pool(name="pv_ps", bufs=1, space="PSUM"))
        tp_ps = s12.enter_context(tc.tile_pool(name="tp_ps", bufs=3, space="PSUM"))

        # super-blocks of up to 512 tokens
        sblocks = []
        t0 = 0
        while t0 < T_kv:
            w = min(512, T_kv - t0)
            sblocks.append((t0, w))
            t0 += w

        for (tb, bw) in sblocks:
            ntile = bw // 128
            for tt in range(ntile):
                ti = tb // 128 + tt
                if ti < 4:
                    x_t = xpre[:, ti, :]
                else:
                    x_t = ln_pool.tile([128, C], BF16, tag="x")
                    nc.sync.dma_start(
                        out=x_t[:],
                        in_=xb_in[ti * 128:(ti + 1) * 128, :])
                xg = x_t.rearrange("p (g d) -> p g d", g=3)
                stats = ln_pool.tile([128, 3, 6], F32, tag="st")
                for g in range(3):
                    nc.vector.bn_stats(out=stats[:, g, :], in_=xg[:, g, :])
                mv = ln_pool.tile([128, 2], F32, tag="mv")
                nc.vector.bn_aggr(out=mv[:], in_=stats[:])
                rstd = ln_pool.tile([128, 1], F32, tag="rstd")
                nc.scalar.activation(
                    out=rstd[:], in_=mv[:, 1:2],
                    func=mybir.ActivationFunctionType.Sqrt,
                    bias=eps_t[:], scale=1.0,
                )
                nc.vector.reciprocal(out=rstd[:], in_=rstd[:])
                h_t = ln_pool.tile([128, C], BF16, tag="h")
                nc.vector.tensor_scalar(
                    out=h_t[:], in0=x_t[:],
                    scalar1=mv[:, 0:1], scalar2=rstd[:],
                    op0=mybir.AluOpType.subtract, op1=mybir.AluOpType.mult,
                )
                pt = tp_ps.tile([128, 6, 128], BF16, tag="tp")
                for c in range(6):
                    nc.tensor.transpose(pt[:, c, :], h_t[:, c * 128:(c + 1) * 128], id_b[:])
                # evict all 6 transposed chunks in one ACT copy (DVE is busy)
                nc.scalar.copy(
                    hT_sb[:, :, ti * 128:(ti + 1) * 128], pt[:],
                )
            # K GEMM for this super-block: kT[:, jj, tb:tb+bw]
            for jj in range(6):
                pk = gemm_ps.tile([128, 512], F32, tag="pk")
                for c in range(6):
                    nc.tensor.matmul(
                        pk[:, 0:bw], wk_sb[:, c, jj * 128:(jj + 1) * 128],
                        hT_sb[:, c, tb:tb + bw],
                        start=(c == 0), stop=(c == 5),
                    )
                if WB:
                    nc.vector.tensor_scalar(
                        out=kT_sb[:, jj, tb:tb + bw], in0=pk[:, 0:bw],
                        scalar1=bk_sb[:, jj:jj + 1], scalar2=None,
                        op0=mybir.AluOpType.add,
                    )
                else:
                    nc.vector.tensor_copy(kT_sb[:, jj, tb:tb + bw], pk[:, 0:bw])
            # V GEMM (natural layout) for this super-block
            for tt in range(ntile):
                ti = tb // 128 + tt
                for half in range(2):
                    h0 = half * 6
                    pv = pv_ps.tile([128, 384], F32, tag=f"pv{half}")
                    for c in range(6):
                        nc.tensor.matmul(
                            pv[:], hT_sb[:, c, ti * 128:(ti + 1) * 128],
                            wv_sb[:, c, half * 384:(half + 1) * 384],
                            start=(c == 0), stop=(c == 5),
                        )
                    vdst = v_sb[:, ti, h0:h0 + 6, 0:64]
                    if WB:
                        nc.vector.tensor_tensor(
                            out=vdst,
                            in0=pv[:].rearrange("p (h d) -> p h d", d=64),
                            in1=bv_bc[:, h0 * 64:(h0 + 6) * 64].rearrange(
                                "p (h d) -> p h d", d=64),
                            op=mybir.AluOpType.add,
                        )
                    else:
                        nc.vector.tensor_copy(
                            vdst, pv[:].rearrange("p (h d) -> p h d", d=64))

        # Q GEMM for own strips (both strips in one N=512 matmul)
        tbA, tbB = sA * 256, sB * 256
        for jj in range(6):
            pq = gemm_ps.tile([128, 512], F32, tag="pk")
            for c in range(6):
                rhs = strided2(hT_sb[:, c, :], tbA, tbB, 256)
                nc.tensor.matmul(
                    pq[:], wq_sb[:, c, jj * 128:(jj + 1) * 128], rhs,
                    start=(c == 0), stop=(c == 5),
                )
            if WB:
                nc.vector.tensor_scalar(
                    out=qT_sb[:, jj, :], in0=pq[:],
                    scalar1=bq_sb[:, jj:jj + 1], scalar2=None,
                    op0=mybir.AluOpType.add,
                )
            else:
                nc.vector.tensor_copy(qT_sb[:, jj, :], pq[:])

        s12.close()  # free wk/wv/wq/hT SBUF

        # prefetch fc weights during attention (into the space s12 freed)
        wmlp = ctx.enter_context(tc.tile_pool(name="wmlp", bufs=1))
        wfc_sb = wmlp.tile([128, 6, DFF], BF16)
        nc.sync.dma_start(out=wfc_sb[:], in_=wfc_in[:].rearrange("(c k) n -> k c n", k=128))

        # ======================= stage 3: attention ========================
        s3 = ExitStack()
        att_pool = s3.enter_context(tc.tile_pool(name="att", bufs=4))
        nrm_pool = s3.enter_context(tc.tile_pool(name="nrm", bufs=2))
        pa_ps = s3.enter_context(tc.tile_pool(name="pa_ps", bufs=3, space="PSUM"))
        yt_ps = s3.enter_context(tc.tile_pool(name="yt_ps", bufs=2, space="PSUM"))

        # Cross-head software pipeline: AV matmuls run PIPE_AV chunks behind
        # the QK/exp frontier, so the PE always has independent QK work (from
        # the next head) queued ahead of exp-dependent AVs.  A small dummy
        # matmul per group tops up PE occupancy so the HAM clock gate stays
        # at full rate through this ACT-bound phase.
        PIPE_AV = 6
        pending = []   # (kc, at_slice, qs, ww, yt_handle, h, is_last)

        def finalize_head(yt, h):
            j, po = h // 2, 64 * (h % 2)
            sume = nrm_pool.tile([1, 512], F32, tag="sume")
            nc.vector.tensor_copy(sume[:], yt[64:65, :])
            bcast = nrm_pool.tile([64, 512], F32, tag="bcast")
            nc.gpsimd.partition_broadcast(bcast[:], sume[:])
            nc.vector.reciprocal_approx_fast(out=bcast[:], in_=bcast[:])
            nc.vector.tensor_mul(
                yT_sb[po:po + 64, j, :], yt[0:64, :], bcast[:],
            )

        def drain(n_keep):
            while len(pending) > n_keep:
                kc, at_sl, qs, ww, yt, h, last = pending.pop(0)
                nc.tensor.matmul(
                    yt[0:65, qs:qs + ww], v_sb[:, kc, h, 0:65],
                    at_sl[:, 0:ww],
                    start=(kc == 0), stop=(kc == n_all - 1),
                    skip_group_check=True,
                )
                if last:
                    finalize_head(yt, h)

        for h in range(12):
            j, po = h // 2, 64 * (h % 2)
            kT_h = kT_sb[po:po + 64, j, :]
            qT_h = qT_sb[po:po + 64, j, :]
            yt = yt_ps.tile([65, 512], F32, tag="yt")

            # shared groups: pairs of kt chunks seen by both strips (q width 512)
            for g in range(n_sh // 2):
                kc0 = 2 * g
                pa = pa_ps.tile([128, 4, 256], F32, tag="pa")
                nc.tensor.matmul(   # HAM-warmth filler; overwritten by QK
                    pa[:, 0:2, :].rearrange("p a b -> p (a b)"),
                    id_b[:], qT_sb[:, 5, :], start=True, stop=True,
                )
                for u in range(2):
                    nc.tensor.matmul(
                        pa[:, 2 * u:2 * u + 2, :].rearrange("p a b -> p (a b)"),
                        kT_h[:, (kc0 + u) * 128:(kc0 + u + 1) * 128],
                        qT_h[:, 0:512], start=True, stop=True,
                    )
                at = att_pool.tile([128, 2, 512], BF16, tag="at2")
                nc.scalar.activation(
                    out=at[:].rearrange("p a b -> p (a b)"),
                    in_=pa[:].rearrange("p a b -> p (a b)"),
                    func=mybir.ActivationFunctionType.Exp)
                for u in range(2):
                    kc = kc0 + u
                    if kc in (2 * sA, 2 * sA + 1):
                        nc.vector.tensor_mul(
                            at[:, u, 0:256], at[:, u, 0:256],
                            mask_t[:, kc - 2 * sA, :])
                pending.append((kc0, at[:, 0, :], 0, 512, yt, h, False))
                pending.append((kc0 + 1, at[:, 1, :], 0, 512, yt, h,
                                kc0 + 1 == n_all - 1))
                drain(PIPE_AV)

            # strip-B-only groups of up to 4 kt chunks (q width 256)
            kc = n_sh
            while kc < n_all:
                gsz = min(4, n_all - kc)
                pa = pa_ps.tile([128, 4, 256], F32, tag="pa")
                nc.tensor.matmul(   # HAM-warmth filler; overwritten by QK
                    pa[:, 0:2, :].rearrange("p a b -> p (a b)"),
                    id_b[:], qT_sb[:, 5, :], start=True, stop=True,
                )
                for u in range(gsz):
                    nc.tensor.matmul(
                        pa[:, u, :], kT_h[:, (kc + u) * 128:(kc + u + 1) * 128],
                        qT_h[:, 256:512], start=True, stop=True,
                    )
                at = att_pool.tile([128, 4, 256], BF16, tag="at4")
                nc.scalar.activation(
                    out=at[:, 0:gsz, :].rearrange("p a b -> p (a b)"),
                    in_=pa[:, 0:gsz, :].rearrange("p a b -> p (a b)"),
                    func=mybir.ActivationFunctionType.Exp)
                for u in range(gsz):
                    if kc + u in (2 * sB, 2 * sB + 1):
                        nc.vector.tensor_mul(
                            at[:, u, :], at[:, u, :],
                            mask_t[:, kc + u - 2 * sB, :])
                for u in range(gsz):
                    pending.append((kc + u, at[:, u, :], 256, 256, yt, h,
                                    kc + u == n_all - 1))
                kc += gsz
                drain(PIPE_AV)
        drain(0)
        s3.close()

        # ============ stage 4: c_proj + residual + LN2 + transpose ==========
        s4 = ExitStack()
        ln2_pool = s4.enter_context(tc.tile_pool(name="ln2", bufs=2))
        cp_ps = s4.enter_context(tc.tile_pool(name="cp_ps", bufs=3, space="PSUM"))
        tp2_ps = s4.enter_context(tc.tile_pool(name="tp2_ps", bufs=2, space="PSUM"))

        def cproj_m(m):
            pp = cp_ps.tile([128, 2, 512], F32, tag="cp")
            for half in range(2):
                for j in range(6):
                    nc.tensor.matmul(
                        pp[:, half, 0:384],
                        yT_sb[:, j, m * 128:(m + 1) * 128],
                        wcp_sb[:, j, half * 384:(half + 1) * 384],
                        start=(j == 0), stop=(j == 5),
                    )
            return pp

        def ln2_m(m, pp):
            x_own = ln2_pool.tile([128, C], F32, tag="xo")
            nc.sync.dma_start(out=x_own[:], in_=xo_in[m * 128:(m + 1) * 128, :])
            if WB:
                nc.vector.tensor_add(x_own[:], x_own[:], bcp_bc[:])
            nc.vector.tensor_add(
                x1_sb[:, m, :].rearrange("p (i n) -> p i n", i=2),
                pp[:, :, 0:384], x_own[:].rearrange("p (i n) -> p i n", i=2),
            )
            x1g = x1_sb[:, m, :].rearrange("p (g d) -> p g d", g=3)
            stats = ln2_pool.tile([128, 3, 6], F32, tag="st2")
            for g in range(3):
                nc.vector.bn_stats(out=stats[:, g, :], in_=x1g[:, g, :])
            mv = ln2_pool.tile([128, 2], F32, tag="mv2")
            nc.vector.bn_aggr(out=mv[:], in_=stats[:])
            rstd = ln2_pool.tile([128, 1], F32, tag="rstd2")
            nc.scalar.activation(
                out=rstd[:], in_=mv[:, 1:2],
                func=mybir.ActivationFunctionType.Sqrt,
                bias=eps_t[:], scale=1.0,
            )
            nc.vector.reciprocal(out=rstd[:], in_=rstd[:])
            h2 = ln2_pool.tile([128, C], BF16, tag="h2")
            nc.vector.tensor_scalar(
                out=h2[:], in0=x1_sb[:, m, :],
                scalar1=mv[:, 0:1], scalar2=rstd[:],
                op0=mybir.AluOpType.subtract, op1=mybir.AluOpType.mult,
            )
            return h2

        def tp2_m(m, h2):
            pt = tp2_ps.tile([128, 6, 128], BF16, tag="tp2")
            for c in range(6):
                nc.tensor.transpose(pt[:, c, :], h2[:, c * 128:(c + 1) * 128], id_b[:])
            nc.scalar.copy(
                h2T_sb[:, :, m * 128:(m + 1) * 128], pt[:],
            )

        # interleave so the DVE LN2 chain of block m runs under the PE
        # c_proj matmuls of later blocks
        pps, h2s = {}, {}
        pps[0] = cproj_m(0)
        pps[1] = cproj_m(1)
        h2s[0] = ln2_m(0, pps[0])
        pps[2] = cproj_m(2)
        h2s[1] = ln2_m(1, pps[1])
        pps[3] = cproj_m(3)
        tp2_m(0, h2s[0])
        h2s[2] = ln2_m(2, pps[2])
        tp2_m(1, h2s[1])
        h2s[3] = ln2_m(3, pps[3])
        tp2_m(2, h2s[2])
        tp2_m(3, h2s[3])
        s4.close()

        # ================== stage 5+6: fc+gelu, proj+store =================
        s56 = ExitStack()
        wpj_pool = s56.enter_context(tc.tile_pool(name="wpj", bufs=6))
        wpj_tiles = []
        for f in range(6):      # prefetch proj weights under the fc GEMMs
            wpj_t = wpj_pool.tile([128, C], BF16, tag="wpj")
            nc.sync.dma_start(out=wpj_t[:], in_=wpj_in[f * 128:(f + 1) * 128, :])
            wpj_tiles.append(wpj_t)

        s5 = ExitStack()
        pf_ps = s5.enter_context(tc.tile_pool(name="pf_ps", bufs=3, space="PSUM"))
        for f in range(24):
            pf = pf_ps.tile([128, 512], F32, tag="pf")
            for c in range(6):
                nc.tensor.matmul(
                    pf[:], wfc_sb[:, c, f * 128:(f + 1) * 128], h2T_sb[:, c, :],
                    start=(c == 0), stop=(c == 5),
                )
            nc.scalar.activation(
                out=gT_sb[:, f, :], in_=pf[:],
                func=mybir.ActivationFunctionType.Gelu_apprx_tanh,
                bias=(bfc_sb[:, f:f + 1] if WB else 0.0), scale=1.0,
            )
        s5.close()

        out_pool = s56.enter_context(tc.tile_pool(name="outp", bufs=3))
        pj_ps = s56.enter_context(tc.tile_pool(name="pj_ps", bufs=1, space="PSUM"))
        pj = []
        for i in range(8):
            pj_i = pj_ps.tile([128, 384], F32, tag=f"pj{i}")
            pj.append(pj_i)
        for f in range(24):
            if f < 6:
                wpj_t = wpj_tiles[f]
            else:
                wpj_t = wpj_pool.tile([128, C], BF16, tag="wpj")
                nc.sync.dma_start(out=wpj_t[:], in_=wpj_in[f * 128:(f + 1) * 128, :])
            for m in range(4):
                for half in range(2):
                    nc.tensor.matmul(
                        pj[m * 2 + half][:],
                        gT_sb[:, f, m * 128:(m + 1) * 128],
                        wpj_t[:, half * 384:(half + 1) * 384],
                        start=(f == 0), stop=(f == 23),
                    )
        for m in range(4):
            o_t = out_pool.tile([128, C], F32, tag="o")
            for half in range(2):
                nc.vector.tensor_add(
                    o_t[:, half * 384:(half + 1) * 384],
                    pj[m * 2 + half][:],
                    x1_sb[:, m, half * 384:(half + 1) * 384],
                )
                if WB:
                    nc.vector.tensor_add(
                        o_t[:, half * 384:(half + 1) * 384],
                        o_t[:, half * 384:(half + 1) * 384],
                        bpj_bc[:, half * 384:(half + 1) * 384],
                    )
                nc.sync.dma_start(
                    out=out_dram[m * 128:(m + 1) * 128, half * 384:(half + 1) * 384],
                    in_=o_t[:, half * 384:(half + 1) * 384])
        s56.close()


# ---------------------------------------------------------------------------
# Runner
# ---------------------------------------------------------------------------
def _make_runner(nc):
    partition_name = nc.partition_id_tensor.name if nc.partition_id_tensor else None
    in_names, out_names, out_avals, zero_outs = [], [], [], []
    for alloc in nc.m.functions[0].allocations:
        if not isinstance(alloc, mybir.MemoryLocationSet):
            continue
        name = alloc.memorylocations[0].name
        if alloc.kind == "ExternalInput":
            if name != partition_name:
                in_names.append(name)
        elif alloc.kind == "ExternalOutput":
            out_names.append(name)
            shape = tuple(alloc.tensor_shape)
            dtype = mybir.dt.np(alloc.dtype)
            out_avals.append(jax.core.ShapedArray(shape, dtype))
            zero_outs.append(np.zeros(shape, dtype))
    n_params = len(in_names)
    all_names = list(in_names) + list(out_names)
    if partition_name is not None:
        all_names.append(partition_name)

    def _body(*args):
        operands = list(args)
        if partition_name is not None:
            operands.append(partition_id_tensor())
        outs = _bass_exec_p.bind(
            *operands,
            out_avals=tuple(out_avals),
            in_names=tuple(all_names),
            out_names=tuple(out_names),
            lowering_input_output_aliases=(),
            sim_require_finite=True,
            sim_require_nnan=True,
            nc=nc,
        )
        return tuple(outs)

    donate = tuple(range(n_params, n_params + len(out_names)))
    jitted = jax.jit(_body, donate_argnums=donate, keep_unused=True)
    return jitted, in_names, out_names, zero_outs


@functools.lru_cache(maxsize=None)
def _get_runners(with_bias: bool):
    install_neuronx_cc_hook()
    runners = []
    for r in range(4):
        nc = build_rank_program(r, with_bias)
        runners.append(_make_runner(nc))
    return runners


def _prep_core_inputs(x, ln1_w, ln1_b, c_attn_w, c_attn_b, c_proj_w, c_proj_b,
                      ln2_w, ln2_b, fc_w, fc_b, proj_w, proj_b):
    """Fold LN affines into weights; split qkv; cast to bf16 on host."""
    import ml_dtypes
    f32 = np.float32
    bf16 = ml_dtypes.bfloat16
    wqkv = (ln1_w[:, None] * c_attn_w).astype(f32)
    bqkv = (c_attn_b + ln1_b @ c_attn_w).astype(f32)
    scale = f32(1.0 / np.sqrt(HD))
    biases = {
        "bq": np.ascontiguousarray(bqkv[0:C] * scale),
        "bk": np.ascontiguousarray(bqkv[C:2 * C]),
        "bv": np.ascontiguousarray(bqkv[2 * C:3 * C]),
        "bcp": np.ascontiguousarray(c_proj_b.astype(f32)),
        "bfc": np.ascontiguousarray((fc_b + ln2_b @ fc_w).astype(f32)),
        "bpj": np.ascontiguousarray(proj_b.astype(f32)),
    }
    with_bias = any(np.any(b != 0) for b in biases.values())
    shared = {
        "wq": np.ascontiguousarray((wqkv[:, 0:C] * scale).astype(bf16)),
        "wk": np.ascontiguousarray(wqkv[:, C:2 * C].astype(bf16)),
        "wv": np.ascontiguousarray(wqkv[:, 2 * C:3 * C].astype(bf16)),
        "wcp": np.ascontiguousarray(c_proj_w.astype(bf16)),
        "wfc": np.ascontiguousarray((ln2_w[:, None] * fc_w).astype(bf16)),
        "wpj": np.ascontiguousarray(proj_w.astype(bf16)),
    }
    if with_bias:
        shared.update(biases)
    return shared, with_bias


def _dispatch_all(inputs):
    """Dispatch the 8 per-core executions asynchronously; return futures."""
    import ml_dtypes
    shared, with_bias = _prep_core_inputs(
        **{k: np.asarray(v) for k, v in inputs.items()})
    runners = _get_runners(with_bias)
    devices = jax.devices()
    x = np.asarray(inputs["x"], dtype=np.float32)
    xb = x.astype(ml_dtypes.bfloat16)
    futs = []
    for c in range(8):
        b, r = c // 4, c % 4
        sA, sB = r, 7 - r
        jitted, in_names, out_names, zero_outs = runners[r]
        dev = devices[c]
        per_core = dict(shared)
        per_core["xb"] = np.ascontiguousarray(xb[b, :256 * (8 - r)])
        per_core["xo"] = np.ascontiguousarray(np.concatenate(
            [x[b, sA * 256:(sA + 1) * 256], x[b, sB * 256:(sB + 1) * 256]]))
        args = [jax.device_put(per_core[n], dev) for n in in_names]
        args += [jax.device_put(z, dev) for z in zero_outs]
        futs.append((c, out_names, jitted(*args)))
    return futs


def kernel(**inputs) -> np.ndarray:
    futs = _dispatch_all(inputs)
    out = np.empty((B, T, C), dtype=np.float32)
    for c, out_names, fut in futs:
        b, r = c // 4, c % 4
        res = np.asarray(fut[out_names.index("out")])
        out[b, 256 * r:256 * r + 256] = res[0:256]
        out[b, 256 * (7 - r):256 * (7 - r) + 256] = res[256:512]
    return out
